# revision 15
# baseline (speedup 1.0000x reference)
"""Chamfer loss kernel for Trainium2 (8 NeuronCores, batch-data-parallel).

Math: for each batch b, dist_sq[n,m] = |p3[n]|^2 + |q3[m]|^2 - 2 p3[n].q3[m].
The reference takes sqrt(max(dist_sq,0)+eps) then dual-axis mins then sums.
sqrt/max/+eps are monotone, so min commutes with them: the device computes
min_m dist_sq (per n) and min_n dist_sq (per m); the host applies
sqrt(max(.,0)+eps) to the 2*B*N mins and sums in float64.

Single-pass device strategy (per core, 16 batches): the distance matrix is
computed ONCE per batch (not once per reduction direction):
  - One K=24 bf16 matmul per (batch, 128-row n-tile, 512-col bank) writes
    PSUM = dist_sq/2 directly (3-level bf16 split of the 3-vectors plus
    norm rows; ~1e-6 abs error). 4 batches ride the 4 PE row-groups.
  - ScalarE (the only PSUM->SBUF stream; ACTIVATE is always 1 elem/cyc)
    evacuates each 4-bank PSUM pair to an fp16 staging buffer s[4r,4g,1024]
    covering 4 rounds (a half-quad: 4 n-tiles x 4 batches).
  - DVE work is batched per half-quad so the ~150ns/op fixed cost
    amortizes over 16 tiles, all fp16 SBUF at 2x_1p:
      col-min (over n, per m): tensor_tensor min tree over the 4 rounds
        (t=min(s0,s1); u=min(s2,s3); min(t,u) -> into / with the
        persistent per-batch accumulator acc[128, 16, 1024]).
      row-min (over m, per n): shared fold chain 1024->512->256->128->64
        ->32 (tensor_tensor) + one 1x tensor_reduce -> res columns
        [4 rounds x 4 batches] per half-quad.
  - Finals: per batch, 8 PE transposes of acc chunks (fp16 -> 1 fp16 PSUM
    bank) + one DVE tensor_reduce (axis=X) -> per-m col-mins, in a second
    PSUM pool phase after the main rounds release all 8 banks.
Output: (128, 256) fp32 per core: col 16*hq + 4*r + g (hq=half-quad 0..7,
r=round-in-hq, g=batch-in-quad) = row-min for batch 4*(hq//2)+g, n-tile
i = 4*(hq%2)+r, lane = n%128; col 128 + 8*b + c = col-min for batch b,
m = 128*c + lane; values are min dist_sq/2. Host decodes and sums.
"""

import numpy as np

import concourse.bass as bass  # noqa: F401  (bass types used via bacc/tile)
import concourse.mybir as mybir
import concourse.tile as tile
from concourse import bacc, masks
from concourse.bass_utils import run_bass_kernel_spmd

B, N, M = 128, 1024, 1024
NCORES = 8
BPC = B // NCORES  # 16 batches per core
NQUAD = BPC // 4  # 4 quads of 4 batches
F32 = mybir.dt.float32
BF16 = mybir.dt.bfloat16
F16 = mybir.dt.float16
KROWS = 24  # bf16 3-level split: 18 cross rows + 3 qn rows + 3 pn rows
MIN = mybir.AluOpType.min

_CACHE = {}


def _body(tc, dram, res_d):
    nc = tc.nc
    with (
        tc.tile_pool(name="stacks", bufs=1) as stacks,
        tc.tile_pool(name="scratchp", bufs=1) as scratchp,
        tc.tile_pool(name="accp", bufs=1) as accp,
        tc.tile_pool(name="resp", bufs=1) as resp,
    ):
        stk = {}
        # host stacks are padded to all 128 partitions so each transfer is
        # ONE descriptor, and each quad gets its OWN tile so a matmul only
        # depends on its quad's transfer. Quad-0 goes first, split across
        # two dispatch queues, so round 0's matmuls start ~8us in.
        for nm in ("ap_s", "bq_s"):
            stk[nm] = [
                stacks.tile([128, 1024], BF16, name=f"{nm}{t}", tag=f"{nm}{t}")
                for t in range(NQUAD)
            ]
        nc.gpsimd.dma_start(out=stk["ap_s"][0], in_=dram["ap_s"][:, 0])
        nc.scalar.dma_start(out=stk["bq_s"][0], in_=dram["bq_s"][:, 0])
        for t in range(1, NQUAD):
            nc.sync.dma_start(out=stk["ap_s"][t], in_=dram["ap_s"][:, t])
            nc.scalar.dma_start(out=stk["bq_s"][t], in_=dram["bq_s"][:, t])

        res_t = resp.tile([128, 256], F32, name="res_t", tag="res_t")
        acc = accp.tile([128, BPC, 1024], F16, name="acc", tag="acc")
        ident = resp.tile([128, 128], F16, name="ident", tag="ident")
        masks.make_identity(nc, ident[:, :])

        A_l, Bs_l = stk["ap_s"], stk["bq_s"]

        with tc.tile_pool(name="psump", bufs=1, space="PSUM") as psump:
            for hq in range(2 * NQUAD):  # half-quads: 4 rounds x 4 batches
                t_i, ihalf = hq // 2, hq % 2
                s = scratchp.tile(
                    [128, 4, 4, 1024], F16, name="s", tag="s", bufs=2
                )
                for r in range(4):
                    i = 4 * ihalf + r
                    pr = [
                        psump.tile([128, 2, 1024], F32, name=f"pr{h}", tag=f"pr{h}")
                        for h in range(2)
                    ]
                    for j in range(2):
                        for g in range(4):
                            nc.tensor.matmul(
                                pr[g // 2][:, g % 2, 512 * j : 512 * (j + 1)],
                                A_l[t_i][
                                    32 * g : 32 * g + KROWS,
                                    128 * i : 128 * (i + 1),
                                ],
                                Bs_l[t_i][
                                    32 * g : 32 * g + KROWS,
                                    512 * j : 512 * (j + 1),
                                ],
                                start=True,
                                stop=True,
                                tile_position=(32 * g, 0),
                            )
                    for h in range(2):
                        nc.scalar.copy(s[:, r, 2 * h : 2 * h + 2, :], pr[h])

                # col-min: fold the 4 rounds elementwise, then into acc
                gp = 4 * t_i
                u0 = scratchp.tile([128, 4, 1024], F16, name="u0", tag="u0", bufs=1)
                u1 = scratchp.tile([128, 4, 1024], F16, name="u1", tag="u1", bufs=1)
                w1 = scratchp.tile([128, 4, 4, 512], F16, name="w1", tag="w1", bufs=1)
                w2 = scratchp.tile([128, 4, 4, 256], F16, name="w2", tag="w2", bufs=1)
                w3 = scratchp.tile([128, 4, 4, 128], F16, name="w3", tag="w3", bufs=2)
                nc.vector.tensor_tensor(out=u0, in0=s[:, 0], in1=s[:, 1], op=MIN)
                if hq == 0:
                    # rounds 0-1 data only: fill the ACT-evacuation ramp
                    nc.vector.tensor_tensor(
                        out=w1[:, 0:2],
                        in0=s[:, 0:2, :, 0:512],
                        in1=s[:, 0:2, :, 512:1024],
                        op=MIN,
                    )
                    nc.vector.tensor_tensor(
                        out=w2[:, 0:2],
                        in0=w1[:, 0:2, :, 0:256],
                        in1=w1[:, 0:2, :, 256:512],
                        op=MIN,
                    )
                    nc.vector.tensor_tensor(
                        out=w3[:, 0:2],
                        in0=w2[:, 0:2, :, 0:128],
                        in1=w2[:, 0:2, :, 128:256],
                        op=MIN,
                    )
                nc.vector.tensor_tensor(out=u1, in0=s[:, 2], in1=s[:, 3], op=MIN)
                if ihalf == 0:
                    nc.vector.tensor_tensor(
                        out=acc[:, gp : gp + 4, :], in0=u0, in1=u1, op=MIN
                    )
                else:
                    nc.vector.tensor_tensor(out=u0, in0=u0, in1=u1, op=MIN)
                    nc.vector.tensor_tensor(
                        out=acc[:, gp : gp + 4, :],
                        in0=u0,
                        in1=acc[:, gp : gp + 4, :],
                        op=MIN,
                    )

                # row-min: shared fold chain + one 1x reduce -> 16 columns
                if hq == 0:
                    nc.vector.tensor_tensor(
                        out=w1[:, 2:4],
                        in0=s[:, 2:4, :, 0:512],
                        in1=s[:, 2:4, :, 512:1024],
                        op=MIN,
                    )
                else:
                    nc.vector.tensor_tensor(
                        out=w1, in0=s[:, :, :, 0:512], in1=s[:, :, :, 512:1024], op=MIN
                    )
                if hq == 0:
                    nc.vector.tensor_tensor(
                        out=w2[:, 2:4],
                        in0=w1[:, 2:4, :, 0:256],
                        in1=w1[:, 2:4, :, 256:512],
                        op=MIN,
                    )
                    nc.vector.tensor_tensor(
                        out=w3[:, 2:4],
                        in0=w2[:, 2:4, :, 0:128],
                        in1=w2[:, 2:4, :, 128:256],
                        op=MIN,
                    )
                else:
                    nc.vector.tensor_tensor(
                        out=w2, in0=w1[:, :, :, 0:256], in1=w1[:, :, :, 256:512], op=MIN
                    )
                    nc.vector.tensor_tensor(
                        out=w3, in0=w2[:, :, :, 0:128], in1=w2[:, :, :, 128:256], op=MIN
                    )
                w4 = scratchp.tile([128, 4, 4, 64], F16, name="w4", tag="w4", bufs=1)
                w5 = scratchp.tile([128, 4, 4, 32], F16, name="w5", tag="w5", bufs=1)
                nc.vector.tensor_tensor(
                    out=w4, in0=w3[:, :, :, 0:64], in1=w3[:, :, :, 64:128], op=MIN
                )
                nc.vector.tensor_tensor(
                    out=w5, in0=w4[:, :, :, 0:32], in1=w4[:, :, :, 32:64], op=MIN
                )
                nc.vector.tensor_reduce(
                    out=res_t[:, 16 * hq : 16 * (hq + 1)],
                    in_=w5,
                    axis=mybir.AxisListType.X,
                    op=MIN,
                )

        # finals: per batch, transpose acc chunks and free-reduce -> col-mins.
        # Two parallel streams: even batches reduce on DVE straight from
        # PSUM; odd batches go ACT-evac (idle ScalarE) -> GPSIMD reduce.
        with tc.tile_pool(name="psumf", bufs=1, space="PSUM") as psumf:
            for g16 in range(BPC):
                ft = psumf.tile([128, 8, 128], F16, name="ft", tag="ft", bufs=4)
                for c in range(8):
                    nc.tensor.transpose(
                        ft[:, c, :], acc[:, g16, 128 * c : 128 * (c + 1)], ident[:, :]
                    )
                nc.vector.tensor_reduce(
                    out=res_t[:, 128 + 8 * g16 : 128 + 8 * (g16 + 1)],
                    in_=ft,
                    axis=mybir.AxisListType.X,
                    op=MIN,
                )

        nc.sync.dma_start(out=res_d, in_=res_t)


def _build_nc():
    if "nc" in _CACHE:
        return _CACHE["nc"]
    nc = bacc.Bacc(
        "TRN2", target_bir_lowering=False, debug=False, num_devices=NCORES
    )
    dram = {}
    for nm in ("ap_s", "bq_s"):
        dram[nm] = nc.dram_tensor(
            nm, (128, NQUAD, 1024), BF16, kind="ExternalInput"
        ).ap()
    res_d = nc.dram_tensor("res", (128, 256), F32, kind="ExternalOutput").ap()
    with tile.TileContext(nc) as tc:
        _body(tc, dram, res_d)
    nc.compile()
    _CACHE["nc"] = nc
    return nc


def _split3(x):
    """Split fp32 into 3 bf16 terms (x ~= h + l + r, error ~2^-27 |x|)."""
    import ml_dtypes

    bf = ml_dtypes.bfloat16
    h = x.astype(bf)
    l = (x - h.astype(np.float32)).astype(bf)
    r = (x - h.astype(np.float32) - l.astype(np.float32)).astype(bf)
    return h, l, r


def _host_stacks(x3, xn, lhs):
    """x3: (BPC, 1024, 3), xn: (BPC, 1024) -> (4, KROWS, NQUAD, 1024) bf16.

    Layout [g, k, t, n]: batch 4*t + g lives in PE row-group g (SBUF
    partitions 32g+k). With s = -x3 for lhsT (s = x3 for rhs) and
    h/l/r the bf16 3-level split, the K pairing slots are
      cross (x3): lhsT [h h l h r l], rhs [h l h r h l]  (x3 comps each)
      norms: lhsT [1 1 1 h(xn/2) l r], rhs [h(yn/2) l r 1 1 1]
    so lhsT[k]*rhs[k] accumulates hh+hl+lh+hr+rh+ll cross terms plus the
    3-term norm halves -> PSUM = dist_sq/2 with ~1e-6 absolute error."""
    import ml_dtypes

    bf = ml_dtypes.bfloat16
    out = np.empty((NQUAD, 4, KROWS, 1024), bf)  # [t, g, k, n]
    sign = -1.0 if lhs else 1.0
    x3t = np.transpose(
        (sign * x3).reshape(NQUAD, 4, 1024, 3), (0, 1, 3, 2)
    )  # (t,g,3,n)
    h3, l3, r3 = _split3(x3t)
    hn, ln, rn = _split3((xn * 0.5).reshape(NQUAD, 4, 1024))
    one = np.asarray(1.0, bf)
    if lhs:
        cross = (h3, h3, l3, h3, r3, l3)
        norm = (one, one, one, hn, ln, rn)
    else:
        cross = (h3, l3, h3, r3, h3, l3)
        norm = (hn, ln, rn, one, one, one)
    for s in range(6):
        out[:, :, 3 * s : 3 * s + 3] = cross[s]
        out[:, :, 18 + s] = norm[s]
    full = np.zeros((4, 32, NQUAD, 1024), bf)  # [g, part-in-group, t, n]
    full[:, :KROWS] = np.transpose(out, (1, 2, 0, 3))
    return np.ascontiguousarray(full.reshape(128, NQUAD, 1024))


def _run(p, q, trace=False, tmpdir=None):
    p = np.asarray(p)
    q = np.asarray(q)
    assert p.shape == (B, N, 4) and q.shape == (B, M, 4)
    p3 = np.ascontiguousarray(p[:, :, 1:], dtype=np.float32)
    q3 = np.ascontiguousarray(q[:, :, 1:], dtype=np.float32)
    pn = np.einsum("bnc,bnc->bn", p3, p3)
    qn = np.einsum("bmc,bmc->bm", q3, q3)

    in_maps = []
    for c in range(NCORES):
        sl = slice(BPC * c, BPC * (c + 1))
        in_maps.append(
            {
                "ap_s": _host_stacks(p3[sl], pn[sl], lhs=True),
                "bq_s": _host_stacks(q3[sl], qn[sl], lhs=False),
            }
        )

    nc = _build_nc()
    kw = {}
    if trace:
        kw = {"trace": True, "tmpdir": tmpdir}
    rb = run_bass_kernel_spmd(nc, in_maps, core_ids=list(range(NCORES)), **kw)

    total = 0.0
    for c in range(NCORES):
        v = 2.0 * rb.results[c]["res"].astype(np.float64)  # (128, 256)
        # all 256 cols hold (independent) mins of dist_sq/2; layout in the
        # module docstring. The sum is layout-independent.
        d_sq = np.maximum(v, 0.0) + 1e-16
        total += np.sqrt(d_sq).sum()
    out = np.float32(total / 2.0)
    return out, rb


def kernel(p, q):
    out, _ = _run(p, q)
    return out


# revision 16
# speedup vs baseline: 1.0057x; 1.0057x over previous
"""Chamfer loss kernel for Trainium2 (8 NeuronCores, batch-data-parallel).

Math: for each batch b, dist_sq[n,m] = |p3[n]|^2 + |q3[m]|^2 - 2 p3[n].q3[m].
The reference takes sqrt(max(dist_sq,0)+eps) then dual-axis mins then sums.
sqrt/max/+eps are monotone, so min commutes with them: the device computes
min_m dist_sq (per n) and min_n dist_sq (per m); the host applies
sqrt(max(.,0)+eps) to the 2*B*N mins and sums in float64.

Single-pass device strategy (per core, 16 batches): the distance matrix is
computed ONCE per batch (not once per reduction direction):
  - One K=24 bf16 matmul per (batch, 128-row n-tile, 512-col bank) writes
    PSUM = dist_sq/2 directly (3-level bf16 split of the 3-vectors plus
    norm rows; ~1e-6 abs error). 4 batches ride the 4 PE row-groups.
  - ScalarE (the only PSUM->SBUF stream; ACTIVATE is always 1 elem/cyc)
    evacuates each 4-bank PSUM pair to an fp16 staging buffer s[4r,4g,1024]
    covering 4 rounds (a half-quad: 4 n-tiles x 4 batches).
  - DVE work is batched per half-quad so the ~150ns/op fixed cost
    amortizes over 16 tiles, all fp16 SBUF at 2x_1p:
      col-min (over n, per m): tensor_tensor min tree over the 4 rounds
        (t=min(s0,s1); u=min(s2,s3); min(t,u) -> into / with the
        persistent per-batch accumulator acc[128, 16, 1024]).
      row-min (over m, per n): shared fold chain 1024->512->256->128->64
        ->32 (tensor_tensor) + one 1x tensor_reduce -> res columns
        [4 rounds x 4 batches] per half-quad.
  - Finals: per batch, 8 PE transposes of acc chunks (fp16 -> 1 fp16 PSUM
    bank) + one DVE tensor_reduce (axis=X) -> per-m col-mins, in a second
    PSUM pool phase after the main rounds release all 8 banks.
Output: (128, 256) fp32 per core: col 16*hq + 4*r + g (hq=half-quad 0..7,
r=round-in-hq, g=batch-in-quad) = row-min for batch 4*(hq//2)+g, n-tile
i = 4*(hq%2)+r, lane = n%128; col 128 + 8*b + c = col-min for batch b,
m = 128*c + lane; values are min dist_sq/2. Host decodes and sums.
"""

import numpy as np

import concourse.bass as bass  # noqa: F401  (bass types used via bacc/tile)
import concourse.mybir as mybir
import concourse.tile as tile
from concourse import bacc, masks
from concourse.bass_utils import run_bass_kernel_spmd

B, N, M = 128, 1024, 1024
NCORES = 8
BPC = B // NCORES  # 16 batches per core
NQUAD = BPC // 4  # 4 quads of 4 batches
F32 = mybir.dt.float32
BF16 = mybir.dt.bfloat16
F16 = mybir.dt.float16
KROWS = 24  # bf16 3-level split: 18 cross rows + 3 qn rows + 3 pn rows
MIN = mybir.AluOpType.min

_CACHE = {}


def _body(tc, dram, res_d):
    nc = tc.nc
    with (
        tc.tile_pool(name="stacks", bufs=1) as stacks,
        tc.tile_pool(name="scratchp", bufs=1) as scratchp,
        tc.tile_pool(name="accp", bufs=1) as accp,
        tc.tile_pool(name="resp", bufs=1) as resp,
    ):
        stk = {}
        # host stacks are padded to all 128 partitions so each transfer is
        # ONE descriptor, and each quad gets its OWN tile so a matmul only
        # depends on its quad's transfer. Quad-0 goes first, split across
        # two dispatch queues, so round 0's matmuls start ~8us in.
        for nm in ("ap_s", "bq_s"):
            stk[nm] = [
                stacks.tile([128, 1024], BF16, name=f"{nm}{t}", tag=f"{nm}{t}")
                for t in range(NQUAD)
            ]
        nc.sync.dma_start(out=stk["ap_s"][0], in_=dram["ap_s"][:, 0])
        nc.scalar.dma_start(out=stk["bq_s"][0], in_=dram["bq_s"][:, 0])
        for t in range(1, NQUAD):
            nc.sync.dma_start(out=stk["ap_s"][t], in_=dram["ap_s"][:, t])
            nc.scalar.dma_start(out=stk["bq_s"][t], in_=dram["bq_s"][:, t])

        res_t = resp.tile([128, 256], F32, name="res_t", tag="res_t")
        acc = accp.tile([128, BPC, 1024], F16, name="acc", tag="acc")
        ident = resp.tile([128, 128], F16, name="ident", tag="ident")
        masks.make_identity(nc, ident[:, :])

        A_l, Bs_l = stk["ap_s"], stk["bq_s"]

        with tc.tile_pool(name="psump", bufs=1, space="PSUM") as psump:
            for hq in range(2 * NQUAD):  # half-quads: 4 rounds x 4 batches
                t_i, ihalf = hq // 2, hq % 2
                s = scratchp.tile(
                    [128, 4, 4, 1024], F16, name="s", tag="s", bufs=2
                )
                for r in range(4):
                    i = 4 * ihalf + r
                    pr = [
                        psump.tile([128, 2, 1024], F32, name=f"pr{h}", tag=f"pr{h}")
                        for h in range(2)
                    ]
                    for j in range(2):
                        for g in range(4):
                            nc.tensor.matmul(
                                pr[g // 2][:, g % 2, 512 * j : 512 * (j + 1)],
                                A_l[t_i][
                                    32 * g : 32 * g + KROWS,
                                    128 * i : 128 * (i + 1),
                                ],
                                Bs_l[t_i][
                                    32 * g : 32 * g + KROWS,
                                    512 * j : 512 * (j + 1),
                                ],
                                start=True,
                                stop=True,
                                tile_position=(32 * g, 0),
                            )
                    for h in range(2):
                        nc.scalar.copy(s[:, r, 2 * h : 2 * h + 2, :], pr[h])

                # col-min: fold the 4 rounds elementwise, then into acc
                gp = 4 * t_i
                u0 = scratchp.tile([128, 4, 1024], F16, name="u0", tag="u0", bufs=1)
                u1 = scratchp.tile([128, 4, 1024], F16, name="u1", tag="u1", bufs=1)
                w1 = scratchp.tile([128, 4, 4, 512], F16, name="w1", tag="w1", bufs=1)
                w2 = scratchp.tile([128, 4, 4, 256], F16, name="w2", tag="w2", bufs=1)
                w3 = scratchp.tile([128, 4, 4, 128], F16, name="w3", tag="w3", bufs=2)
                nc.vector.tensor_tensor(out=u0, in0=s[:, 0], in1=s[:, 1], op=MIN)
                if hq == 0:
                    # rounds 0-1 data only: fill the ACT-evacuation ramp
                    nc.vector.tensor_tensor(
                        out=w1[:, 0:2],
                        in0=s[:, 0:2, :, 0:512],
                        in1=s[:, 0:2, :, 512:1024],
                        op=MIN,
                    )
                    nc.vector.tensor_tensor(
                        out=w2[:, 0:2],
                        in0=w1[:, 0:2, :, 0:256],
                        in1=w1[:, 0:2, :, 256:512],
                        op=MIN,
                    )
                    nc.vector.tensor_tensor(
                        out=w3[:, 0:2],
                        in0=w2[:, 0:2, :, 0:128],
                        in1=w2[:, 0:2, :, 128:256],
                        op=MIN,
                    )
                nc.vector.tensor_tensor(out=u1, in0=s[:, 2], in1=s[:, 3], op=MIN)
                if ihalf == 0:
                    nc.vector.tensor_tensor(
                        out=acc[:, gp : gp + 4, :], in0=u0, in1=u1, op=MIN
                    )
                else:
                    nc.vector.tensor_tensor(out=u0, in0=u0, in1=u1, op=MIN)
                    nc.vector.tensor_tensor(
                        out=acc[:, gp : gp + 4, :],
                        in0=u0,
                        in1=acc[:, gp : gp + 4, :],
                        op=MIN,
                    )

                # row-min: shared fold chain + one 1x reduce -> 16 columns
                if hq == 0:
                    nc.vector.tensor_tensor(
                        out=w1[:, 2:4],
                        in0=s[:, 2:4, :, 0:512],
                        in1=s[:, 2:4, :, 512:1024],
                        op=MIN,
                    )
                else:
                    nc.vector.tensor_tensor(
                        out=w1, in0=s[:, :, :, 0:512], in1=s[:, :, :, 512:1024], op=MIN
                    )
                if hq == 0:
                    nc.vector.tensor_tensor(
                        out=w2[:, 2:4],
                        in0=w1[:, 2:4, :, 0:256],
                        in1=w1[:, 2:4, :, 256:512],
                        op=MIN,
                    )
                    nc.vector.tensor_tensor(
                        out=w3[:, 2:4],
                        in0=w2[:, 2:4, :, 0:128],
                        in1=w2[:, 2:4, :, 128:256],
                        op=MIN,
                    )
                else:
                    nc.vector.tensor_tensor(
                        out=w2, in0=w1[:, :, :, 0:256], in1=w1[:, :, :, 256:512], op=MIN
                    )
                    nc.vector.tensor_tensor(
                        out=w3, in0=w2[:, :, :, 0:128], in1=w2[:, :, :, 128:256], op=MIN
                    )
                w4 = scratchp.tile([128, 4, 4, 64], F16, name="w4", tag="w4", bufs=1)
                w5 = scratchp.tile([128, 4, 4, 32], F16, name="w5", tag="w5", bufs=1)
                nc.vector.tensor_tensor(
                    out=w4, in0=w3[:, :, :, 0:64], in1=w3[:, :, :, 64:128], op=MIN
                )
                nc.vector.tensor_tensor(
                    out=w5, in0=w4[:, :, :, 0:32], in1=w4[:, :, :, 32:64], op=MIN
                )
                nc.vector.tensor_reduce(
                    out=res_t[:, 16 * hq : 16 * (hq + 1)],
                    in_=w5,
                    axis=mybir.AxisListType.X,
                    op=MIN,
                )

        # finals: per batch, transpose acc chunks and free-reduce -> col-mins.
        # Two parallel streams: even batches reduce on DVE straight from
        # PSUM; odd batches go ACT-evac (idle ScalarE) -> GPSIMD reduce.
        with tc.tile_pool(name="psumf", bufs=1, space="PSUM") as psumf:
            for g16 in range(BPC):
                ft = psumf.tile([128, 8, 128], F16, name="ft", tag="ft", bufs=4)
                for c in range(8):
                    nc.tensor.transpose(
                        ft[:, c, :], acc[:, g16, 128 * c : 128 * (c + 1)], ident[:, :]
                    )
                nc.vector.tensor_reduce(
                    out=res_t[:, 128 + 8 * g16 : 128 + 8 * (g16 + 1)],
                    in_=ft,
                    axis=mybir.AxisListType.X,
                    op=MIN,
                )

        nc.sync.dma_start(out=res_d, in_=res_t)


def _build_nc():
    if "nc" in _CACHE:
        return _CACHE["nc"]
    nc = bacc.Bacc(
        "TRN2", target_bir_lowering=False, debug=False, num_devices=NCORES
    )
    dram = {}
    for nm in ("ap_s", "bq_s"):
        dram[nm] = nc.dram_tensor(
            nm, (128, NQUAD, 1024), BF16, kind="ExternalInput"
        ).ap()
    res_d = nc.dram_tensor("res", (128, 256), F32, kind="ExternalOutput").ap()
    with tile.TileContext(nc) as tc:
        _body(tc, dram, res_d)
    nc.compile()
    _CACHE["nc"] = nc
    return nc


def _split3(x):
    """Split fp32 into 3 bf16 terms (x ~= h + l + r, error ~2^-27 |x|)."""
    import ml_dtypes

    bf = ml_dtypes.bfloat16
    h = x.astype(bf)
    l = (x - h.astype(np.float32)).astype(bf)
    r = (x - h.astype(np.float32) - l.astype(np.float32)).astype(bf)
    return h, l, r


def _host_stacks(x3, xn, lhs):
    """x3: (BPC, 1024, 3), xn: (BPC, 1024) -> (4, KROWS, NQUAD, 1024) bf16.

    Layout [g, k, t, n]: batch 4*t + g lives in PE row-group g (SBUF
    partitions 32g+k). With s = -x3 for lhsT (s = x3 for rhs) and
    h/l/r the bf16 3-level split, the K pairing slots are
      cross (x3): lhsT [h h l h r l], rhs [h l h r h l]  (x3 comps each)
      norms: lhsT [1 1 1 h(xn/2) l r], rhs [h(yn/2) l r 1 1 1]
    so lhsT[k]*rhs[k] accumulates hh+hl+lh+hr+rh+ll cross terms plus the
    3-term norm halves -> PSUM = dist_sq/2 with ~1e-6 absolute error."""
    import ml_dtypes

    bf = ml_dtypes.bfloat16
    out = np.empty((NQUAD, 4, KROWS, 1024), bf)  # [t, g, k, n]
    sign = -1.0 if lhs else 1.0
    x3t = np.transpose(
        (sign * x3).reshape(NQUAD, 4, 1024, 3), (0, 1, 3, 2)
    )  # (t,g,3,n)
    h3, l3, r3 = _split3(x3t)
    hn, ln, rn = _split3((xn * 0.5).reshape(NQUAD, 4, 1024))
    one = np.asarray(1.0, bf)
    if lhs:
        cross = (h3, h3, l3, h3, r3, l3)
        norm = (one, one, one, hn, ln, rn)
    else:
        cross = (h3, l3, h3, r3, h3, l3)
        norm = (hn, ln, rn, one, one, one)
    for s in range(6):
        out[:, :, 3 * s : 3 * s + 3] = cross[s]
        out[:, :, 18 + s] = norm[s]
    full = np.zeros((4, 32, NQUAD, 1024), bf)  # [g, part-in-group, t, n]
    full[:, :KROWS] = np.transpose(out, (1, 2, 0, 3))
    return np.ascontiguousarray(full.reshape(128, NQUAD, 1024))


def _run(p, q, trace=False, tmpdir=None):
    p = np.asarray(p)
    q = np.asarray(q)
    assert p.shape == (B, N, 4) and q.shape == (B, M, 4)
    p3 = np.ascontiguousarray(p[:, :, 1:], dtype=np.float32)
    q3 = np.ascontiguousarray(q[:, :, 1:], dtype=np.float32)
    pn = np.einsum("bnc,bnc->bn", p3, p3)
    qn = np.einsum("bmc,bmc->bm", q3, q3)

    in_maps = []
    for c in range(NCORES):
        sl = slice(BPC * c, BPC * (c + 1))
        in_maps.append(
            {
                "ap_s": _host_stacks(p3[sl], pn[sl], lhs=True),
                "bq_s": _host_stacks(q3[sl], qn[sl], lhs=False),
            }
        )

    nc = _build_nc()
    kw = {}
    if trace:
        kw = {"trace": True, "tmpdir": tmpdir}
    rb = run_bass_kernel_spmd(nc, in_maps, core_ids=list(range(NCORES)), **kw)

    total = 0.0
    for c in range(NCORES):
        v = 2.0 * rb.results[c]["res"].astype(np.float64)  # (128, 256)
        # all 256 cols hold (independent) mins of dist_sq/2; layout in the
        # module docstring. The sum is layout-independent.
        d_sq = np.maximum(v, 0.0) + 1e-16
        total += np.sqrt(d_sq).sum()
    out = np.float32(total / 2.0)
    return out, rb


def kernel(p, q):
    out, _ = _run(p, q)
    return out


# revision 17
# speedup vs baseline: 1.0067x; 1.0009x over previous
"""Chamfer loss kernel for Trainium2 (8 NeuronCores, batch-data-parallel).

Math: for each batch b, dist_sq[n,m] = |p3[n]|^2 + |q3[m]|^2 - 2 p3[n].q3[m].
The reference takes sqrt(max(dist_sq,0)+eps) then dual-axis mins then sums.
sqrt/max/+eps are monotone, so min commutes with them: the device computes
min_m dist_sq (per n) and min_n dist_sq (per m); the host applies
sqrt(max(.,0)+eps) to the 2*B*N mins and sums in float64.

Single-pass device strategy (per core, 16 batches): the distance matrix is
computed ONCE per batch (not once per reduction direction):
  - One K=24 bf16 matmul per (batch, 128-row n-tile, 512-col bank) writes
    PSUM = dist_sq/2 directly (3-level bf16 split of the 3-vectors plus
    norm rows; ~1e-6 abs error). 4 batches ride the 4 PE row-groups.
  - ScalarE (the only PSUM->SBUF stream; ACTIVATE is always 1 elem/cyc)
    evacuates each 4-bank PSUM pair to an fp16 staging buffer s[4r,4g,1024]
    covering 4 rounds (a half-quad: 4 n-tiles x 4 batches).
  - DVE work is batched per half-quad so the ~150ns/op fixed cost
    amortizes over 16 tiles, all fp16 SBUF at 2x_1p:
      col-min (over n, per m): tensor_tensor min tree over the 4 rounds
        (t=min(s0,s1); u=min(s2,s3); min(t,u) -> into / with the
        persistent per-batch accumulator acc[128, 16, 1024]).
      row-min (over m, per n): shared fold chain 1024->512->256->128->64
        ->32 (tensor_tensor) + one 1x tensor_reduce -> res columns
        [4 rounds x 4 batches] per half-quad.
  - Finals: per batch, 8 PE transposes of acc chunks (fp16 -> 1 fp16 PSUM
    bank) + one DVE tensor_reduce (axis=X) -> per-m col-mins, in a second
    PSUM pool phase after the main rounds release all 8 banks.
Output: (128, 256) fp32 per core: col 16*hq + 4*r + g (hq=half-quad 0..7,
r=round-in-hq, g=batch-in-quad) = row-min for batch 4*(hq//2)+g, n-tile
i = 4*(hq%2)+r, lane = n%128; col 128 + 8*b + c = col-min for batch b,
m = 128*c + lane; values are min dist_sq/2. Host decodes and sums.
"""

import numpy as np

import concourse.bass as bass  # noqa: F401  (bass types used via bacc/tile)
import concourse.mybir as mybir
import concourse.tile as tile
from concourse import bacc, masks
from concourse.bass_utils import run_bass_kernel_spmd

B, N, M = 128, 1024, 1024
NCORES = 8
BPC = B // NCORES  # 16 batches per core
NQUAD = BPC // 4  # 4 quads of 4 batches
F32 = mybir.dt.float32
BF16 = mybir.dt.bfloat16
F16 = mybir.dt.float16
KROWS = 24  # bf16 3-level split: 18 cross rows + 3 qn rows + 3 pn rows
MIN = mybir.AluOpType.min

_CACHE = {}


def _body(tc, dram, res_d):
    nc = tc.nc
    with (
        tc.tile_pool(name="stacks", bufs=1) as stacks,
        tc.tile_pool(name="scratchp", bufs=1) as scratchp,
        tc.tile_pool(name="accp", bufs=1) as accp,
        tc.tile_pool(name="resp", bufs=1) as resp,
    ):
        stk = {}
        # host stacks are padded to all 128 partitions so each transfer is
        # ONE descriptor, and each quad gets its OWN tile so a matmul only
        # depends on its quad's transfer. Quad-0 goes first, split across
        # two dispatch queues, so round 0's matmuls start ~8us in.
        for nm in ("ap_s", "bq_s"):
            stk[nm] = [
                stacks.tile([128, 1024], BF16, name=f"{nm}{t}", tag=f"{nm}{t}")
                for t in range(NQUAD)
            ]
        nc.sync.dma_start(out=stk["ap_s"][0], in_=dram["ap_s"][:, 0])
        nc.scalar.dma_start(out=stk["bq_s"][0], in_=dram["bq_s"][:, 0])
        for t in range(1, NQUAD):
            nc.sync.dma_start(out=stk["ap_s"][t], in_=dram["ap_s"][:, t])
            nc.scalar.dma_start(out=stk["bq_s"][t], in_=dram["bq_s"][:, t])

        res_t = resp.tile([128, 256], F32, name="res_t", tag="res_t")
        acc = accp.tile([128, BPC, 1024], F16, name="acc", tag="acc")
        ident = resp.tile([128, 128], F16, name="ident", tag="ident")
        masks.make_identity(nc, ident[:, :])

        A_l, Bs_l = stk["ap_s"], stk["bq_s"]

        with tc.tile_pool(name="psump", bufs=1, space="PSUM") as psump:
            for hq in range(2 * NQUAD):  # half-quads: 4 rounds x 4 batches
                t_i, ihalf = hq // 2, hq % 2
                s = scratchp.tile(
                    [128, 4, 4, 1024], F16, name="s", tag="s", bufs=2
                )
                for r in range(4):
                    i = 4 * ihalf + r
                    pr = [
                        psump.tile([128, 2, 1024], F32, name=f"pr{h}", tag=f"pr{h}")
                        for h in range(2)
                    ]
                    for j in range(2):
                        for g in range(4):
                            nc.tensor.matmul(
                                pr[g // 2][:, g % 2, 512 * j : 512 * (j + 1)],
                                A_l[t_i][
                                    32 * g : 32 * g + KROWS,
                                    128 * i : 128 * (i + 1),
                                ],
                                Bs_l[t_i][
                                    32 * g : 32 * g + KROWS,
                                    512 * j : 512 * (j + 1),
                                ],
                                start=True,
                                stop=True,
                                tile_position=(32 * g, 0),
                            )
                    for h in range(2):
                        nc.scalar.copy(s[:, r, 2 * h : 2 * h + 2, :], pr[h])

                # col-min: fold the 4 rounds elementwise, then into acc
                gp = 4 * t_i
                u0 = scratchp.tile([128, 4, 1024], F16, name="u0", tag="u0", bufs=1)
                u1 = scratchp.tile([128, 4, 1024], F16, name="u1", tag="u1", bufs=1)
                w1 = scratchp.tile([128, 4, 4, 512], F16, name="w1", tag="w1", bufs=1)
                w2 = scratchp.tile([128, 4, 4, 256], F16, name="w2", tag="w2", bufs=1)
                w3 = scratchp.tile([128, 4, 4, 128], F16, name="w3", tag="w3", bufs=2)
                nc.vector.tensor_tensor(out=u0, in0=s[:, 0], in1=s[:, 1], op=MIN)
                if hq == 0:
                    # rounds 0-1 data only: fill the ACT-evacuation ramp
                    nc.vector.tensor_tensor(
                        out=w1[:, 0:2],
                        in0=s[:, 0:2, :, 0:512],
                        in1=s[:, 0:2, :, 512:1024],
                        op=MIN,
                    )
                    nc.vector.tensor_tensor(
                        out=w2[:, 0:2],
                        in0=w1[:, 0:2, :, 0:256],
                        in1=w1[:, 0:2, :, 256:512],
                        op=MIN,
                    )
                    nc.vector.tensor_tensor(
                        out=w3[:, 0:2],
                        in0=w2[:, 0:2, :, 0:128],
                        in1=w2[:, 0:2, :, 128:256],
                        op=MIN,
                    )
                nc.vector.tensor_tensor(out=u1, in0=s[:, 2], in1=s[:, 3], op=MIN)
                if ihalf == 0:
                    nc.vector.tensor_tensor(
                        out=acc[:, gp : gp + 4, :], in0=u0, in1=u1, op=MIN
                    )
                else:
                    nc.vector.tensor_tensor(out=u0, in0=u0, in1=u1, op=MIN)
                    nc.vector.tensor_tensor(
                        out=acc[:, gp : gp + 4, :],
                        in0=u0,
                        in1=acc[:, gp : gp + 4, :],
                        op=MIN,
                    )

                # row-min: shared fold chain + one 1x reduce -> 16 columns
                if hq == 0:
                    nc.vector.tensor_tensor(
                        out=w1[:, 2:4],
                        in0=s[:, 2:4, :, 0:512],
                        in1=s[:, 2:4, :, 512:1024],
                        op=MIN,
                    )
                else:
                    nc.vector.tensor_tensor(
                        out=w1, in0=s[:, :, :, 0:512], in1=s[:, :, :, 512:1024], op=MIN
                    )
                if hq == 0:
                    nc.vector.tensor_tensor(
                        out=w2[:, 2:4],
                        in0=w1[:, 2:4, :, 0:256],
                        in1=w1[:, 2:4, :, 256:512],
                        op=MIN,
                    )
                    nc.vector.tensor_tensor(
                        out=w3[:, 2:4],
                        in0=w2[:, 2:4, :, 0:128],
                        in1=w2[:, 2:4, :, 128:256],
                        op=MIN,
                    )
                else:
                    nc.vector.tensor_tensor(
                        out=w2, in0=w1[:, :, :, 0:256], in1=w1[:, :, :, 256:512], op=MIN
                    )
                    nc.vector.tensor_tensor(
                        out=w3, in0=w2[:, :, :, 0:128], in1=w2[:, :, :, 128:256], op=MIN
                    )
                w4 = scratchp.tile([128, 4, 4, 64], F16, name="w4", tag="w4", bufs=1)
                w5 = scratchp.tile([128, 4, 4, 32], F16, name="w5", tag="w5", bufs=1)
                nc.vector.tensor_tensor(
                    out=w4, in0=w3[:, :, :, 0:64], in1=w3[:, :, :, 64:128], op=MIN
                )
                nc.vector.tensor_tensor(
                    out=w5, in0=w4[:, :, :, 0:32], in1=w4[:, :, :, 32:64], op=MIN
                )
                nc.vector.tensor_reduce(
                    out=res_t[:, 16 * hq : 16 * (hq + 1)],
                    in_=w5,
                    axis=mybir.AxisListType.X,
                    op=MIN,
                )

        nc.sync.dma_start(out=res_d[:, 0:128], in_=res_t[:, 0:128])

        # finals: per batch, transpose acc chunks and free-reduce -> col-mins.
        # Two parallel streams: even batches reduce on DVE straight from
        # PSUM; odd batches go ACT-evac (idle ScalarE) -> GPSIMD reduce.
        with tc.tile_pool(name="psumf", bufs=1, space="PSUM") as psumf:
            for g16 in range(BPC):
                ft = psumf.tile([128, 8, 128], F16, name="ft", tag="ft", bufs=4)
                for c in range(8):
                    nc.tensor.transpose(
                        ft[:, c, :], acc[:, g16, 128 * c : 128 * (c + 1)], ident[:, :]
                    )
                nc.vector.tensor_reduce(
                    out=res_t[:, 128 + 8 * g16 : 128 + 8 * (g16 + 1)],
                    in_=ft,
                    axis=mybir.AxisListType.X,
                    op=MIN,
                )

        nc.sync.dma_start(out=res_d[:, 128:256], in_=res_t[:, 128:256])


def _build_nc():
    if "nc" in _CACHE:
        return _CACHE["nc"]
    nc = bacc.Bacc(
        "TRN2", target_bir_lowering=False, debug=False, num_devices=NCORES
    )
    dram = {}
    for nm in ("ap_s", "bq_s"):
        dram[nm] = nc.dram_tensor(
            nm, (128, NQUAD, 1024), BF16, kind="ExternalInput"
        ).ap()
    res_d = nc.dram_tensor("res", (128, 256), F32, kind="ExternalOutput").ap()
    with tile.TileContext(nc) as tc:
        _body(tc, dram, res_d)
    nc.compile()
    _CACHE["nc"] = nc
    return nc


def _split3(x):
    """Split fp32 into 3 bf16 terms (x ~= h + l + r, error ~2^-27 |x|)."""
    import ml_dtypes

    bf = ml_dtypes.bfloat16
    h = x.astype(bf)
    l = (x - h.astype(np.float32)).astype(bf)
    r = (x - h.astype(np.float32) - l.astype(np.float32)).astype(bf)
    return h, l, r


def _host_stacks(x3, xn, lhs):
    """x3: (BPC, 1024, 3), xn: (BPC, 1024) -> (4, KROWS, NQUAD, 1024) bf16.

    Layout [g, k, t, n]: batch 4*t + g lives in PE row-group g (SBUF
    partitions 32g+k). With s = -x3 for lhsT (s = x3 for rhs) and
    h/l/r the bf16 3-level split, the K pairing slots are
      cross (x3): lhsT [h h l h r l], rhs [h l h r h l]  (x3 comps each)
      norms: lhsT [1 1 1 h(xn/2) l r], rhs [h(yn/2) l r 1 1 1]
    so lhsT[k]*rhs[k] accumulates hh+hl+lh+hr+rh+ll cross terms plus the
    3-term norm halves -> PSUM = dist_sq/2 with ~1e-6 absolute error."""
    import ml_dtypes

    bf = ml_dtypes.bfloat16
    out = np.empty((NQUAD, 4, KROWS, 1024), bf)  # [t, g, k, n]
    sign = -1.0 if lhs else 1.0
    x3t = np.transpose(
        (sign * x3).reshape(NQUAD, 4, 1024, 3), (0, 1, 3, 2)
    )  # (t,g,3,n)
    h3, l3, r3 = _split3(x3t)
    hn, ln, rn = _split3((xn * 0.5).reshape(NQUAD, 4, 1024))
    one = np.asarray(1.0, bf)
    if lhs:
        cross = (h3, h3, l3, h3, r3, l3)
        norm = (one, one, one, hn, ln, rn)
    else:
        cross = (h3, l3, h3, r3, h3, l3)
        norm = (hn, ln, rn, one, one, one)
    for s in range(6):
        out[:, :, 3 * s : 3 * s + 3] = cross[s]
        out[:, :, 18 + s] = norm[s]
    full = np.zeros((4, 32, NQUAD, 1024), bf)  # [g, part-in-group, t, n]
    full[:, :KROWS] = np.transpose(out, (1, 2, 0, 3))
    return np.ascontiguousarray(full.reshape(128, NQUAD, 1024))


def _run(p, q, trace=False, tmpdir=None):
    p = np.asarray(p)
    q = np.asarray(q)
    assert p.shape == (B, N, 4) and q.shape == (B, M, 4)
    p3 = np.ascontiguousarray(p[:, :, 1:], dtype=np.float32)
    q3 = np.ascontiguousarray(q[:, :, 1:], dtype=np.float32)
    pn = np.einsum("bnc,bnc->bn", p3, p3)
    qn = np.einsum("bmc,bmc->bm", q3, q3)

    in_maps = []
    for c in range(NCORES):
        sl = slice(BPC * c, BPC * (c + 1))
        in_maps.append(
            {
                "ap_s": _host_stacks(p3[sl], pn[sl], lhs=True),
                "bq_s": _host_stacks(q3[sl], qn[sl], lhs=False),
            }
        )

    nc = _build_nc()
    kw = {}
    if trace:
        kw = {"trace": True, "tmpdir": tmpdir}
    rb = run_bass_kernel_spmd(nc, in_maps, core_ids=list(range(NCORES)), **kw)

    total = 0.0
    for c in range(NCORES):
        v = 2.0 * rb.results[c]["res"].astype(np.float64)  # (128, 256)
        # all 256 cols hold (independent) mins of dist_sq/2; layout in the
        # module docstring. The sum is layout-independent.
        d_sq = np.maximum(v, 0.0) + 1e-16
        total += np.sqrt(d_sq).sum()
    out = np.float32(total / 2.0)
    return out, rb


def kernel(p, q):
    out, _ = _run(p, q)
    return out
